# revision 1
# baseline (speedup 1.0000x reference)
"""Trainium2 Bass kernel for nn_DetectionLoss.

Reference computation:
  cls_loss = mean(softplus(x)) - sum(x at occupied cells)/BHW     (BCE-with-logits)
  reg_loss = sum(smoothl1(reg - target) at occupied cells)/num_objects
  total    = cls_loss + 2*reg_loss ; also returns num_objects

Key insight: only the cls channel (B,H,W) needs a dense pass; the 7 reg
channels are needed at just the <=1024 scattered target cells, so they are
fetched with one 128-row indirect DMA per core instead of reading 7/8 of the
input (8x traffic reduction). Sharding: data-parallel over B, 2 batches per
core; each core emits per-partition partials and the host finishes the tiny
scalar reduction.

Index semantics replicate the neuron backend the reference runs on:
  - f32->int32 conversion rounds to nearest (verified on device); emulated
    here in f32 arithmetic with the +-2^23 trick so it holds exactly
  - scatter .at[].set with duplicate indices: last write wins (verified)

softplus(x) is computed directly as Ln(1 + Exp(x)): preds are N(0,1) logits
so Exp cannot overflow, and the two table functions share one ACT table set
(enforced below) so only a single table load is paid.
"""

import numpy as np

import concourse.bass as bass
import concourse.tile as tile
from concourse import bacc, mybir
from concourse.bass_utils import run_bass_kernel_spmd
from concourse.tile_rust import add_dep_helper

P = 128
B, C, H, W = 16, 8, 512, 512
N_TGT = 64
NCORES = 8
BPC = B // NCORES            # batches per core
CELLS = H * W                # 262144
CORE_ELEMS = BPC * CELLS     # 524288
FREE = CORE_ELEMS // P       # 4096
SPLITS = (672, 1312, 2112)   # dense chunk widths: small first so ACT starts
                             # early, large last (tuned on the cost model)
SLOTS = BPC * N_TGT          # 128 target slots per core
TWO23 = 8388608.0            # 2^23: (x + 2^23) - 2^23 == rint(x), 0<=x<2^23

f32 = mybir.dt.float32
i32 = mybir.dt.int32
ALU = mybir.AluOpType
ACT = mybir.ActivationFunctionType

NCHUNK = len(SPLITS)
COL_X = NCHUNK               # winner_mask * cls_value at cell
COL_M = NCHUNK + 1           # winner mask (1 per unique occupied cell)
COL_REG = NCHUNK + 2         # winner_mask * smoothl1 row sum
OUT_COLS = NCHUNK + 3

_compiled = None
_tables_patched = False


def _stub_axon_hooks():
    """run_bass_kernel_spmd(trace=True) — reachable via the BASS_TRACE env
    var — imports antenv.axon_hooks, which doesn't exist in this container.
    Register a stub whose hook getter returns None so the call degrades to
    an untraced run (bass_utils handles the None hook) instead of crashing."""
    import importlib
    import sys
    import types as _types

    try:
        importlib.import_module("antenv.axon_hooks")
    except Exception:
        m = _types.ModuleType("antenv.axon_hooks")
        m.get_axon_ntff_profile_hook = lambda: None
        sys.modules["antenv.axon_hooks"] = m


_stub_axon_hooks()


def _patch_act_tables():
    """Make Exp and Ln resolve only to the table set that contains both, so
    Bacc's greedy chooser emits a single ACT table load instead of two."""
    global _tables_patched
    if _tables_patched:
        return
    _tables_patched = True
    import concourse.hw_specs as hws

    orig = hws.get_activation_tables

    def patched(arch):
        tables = orig(arch)
        combo = tables.get("natural_log_exp_and_others")
        if combo and ACT.Exp in combo and ACT.Ln in combo:
            # safe to steer: the combined set can serve both funcs
            for name, funcs in tables.items():
                if name != "natural_log_exp_and_others":
                    funcs.discard(ACT.Exp)
                    funcs.discard(ACT.Ln)
        return tables

    hws.get_activation_tables = patched
    bacc.get_activation_tables = patched


def _build():
    _patch_act_tables()
    nc = bacc.Bacc(
        "TRN2", target_bir_lowering=False, debug=False, num_devices=NCORES
    )
    cls_in = nc.declare_dram_parameter("cls", [P, FREE + 8], f32, isOutput=False)
    cl8_in = nc.declare_dram_parameter("cl8", [CORE_ELEMS, C], f32, isOutput=False)
    out_d = nc.declare_dram_parameter("out", [P, OUT_COLS], f32, isOutput=True)
    fc_sc = nc.dram_tensor("fc_scratch", [1, P], f32)

    with tile.TileContext(nc) as tc:
        with tc.tile_pool(name="sbuf", bufs=1) as sp:
            out_t = sp.tile([P, OUT_COLS], f32)

            # ---------------- dense pass: sum softplus(cls) ----------------
            # chunk 0 also carries the 8 targets columns (cols FREE..FREE+7 of
            # the cls input, appended by the host) so the tiny targets load
            # doesn't need its own DMA slot in the stream queue.
            # tg cols 0..6: target values; col 7: batch offset b*CELLS.
            tg = None
            prev_ln = None
            col0 = 0
            for k, wdt in enumerate(SPLITS):
                sl = slice(col0, col0 + wdt)
                col0 += wdt
                if k == 0:
                    # host interleaves the 8 tg columns right after chunk 0,
                    # so one contiguous DMA carries both
                    xt0 = sp.tile([P, wdt + 8], f32, tag="xt0")
                    nc.sync.dma_start(
                        out=xt0[:], in_=cls_in[:, 0 : wdt + 8]
                    )
                    tg = xt0[:, wdt : wdt + 8]
                    xt = xt0[:, 0:wdt]
                    col0 += 8  # later chunks shifted by the inserted tg cols
                else:
                    xt = sp.tile([P, wdt], f32, tag=f"xt{k}")
                    nc.sync.dma_start(out=xt[:], in_=cls_in[:, sl])
                e = nc.scalar.activation(out=xt[:], in_=xt[:], func=ACT.Exp)
                if prev_ln is not None:
                    # keep ACT in per-chunk Exp/Ln order so earlier chunks
                    # finish while later chunks are still streaming in
                    add_dep_helper(e.ins, prev_ln.ins, reason="act order")
                prev_ln = nc.scalar.activation(
                    out=xt[:], in_=xt[:], func=ACT.Ln, bias=1.0,
                    accum_out=out_t[:, k : k + 1],
                )

            # ---------------- target indices (one slot per partition) -------
            def grid_coord(col):
                # rint(clip(t * (512/80), 0, 511)); the rounding must happen
                # in f32 (the +2^23 trick) to mirror the backend's
                # round-to-nearest float->int conversion.
                g = sp.tile([P, 1], f32, tag=f"g{col}")
                nc.vector.tensor_scalar(
                    out=g[:], in0=tg[:, col : col + 1],
                    scalar1=float(np.float32(W / 80.0)), scalar2=511.0,
                    op0=ALU.mult, op1=ALU.min,
                )
                nc.vector.tensor_scalar(
                    out=g[:], in0=g[:], scalar1=0.0, scalar2=None, op0=ALU.max
                )
                gr = sp.tile([P, 1], f32, tag=f"gr{col}")
                nc.vector.tensor_scalar(
                    out=gr[:], in0=g[:], scalar1=TWO23, scalar2=None, op0=ALU.add
                )
                nc.vector.tensor_scalar(
                    out=gr[:], in0=gr[:], scalar1=TWO23, scalar2=None,
                    op0=ALU.subtract,
                )
                return gr

            gx = grid_coord(0)
            gy = grid_coord(1)

            # fc = b*CELLS + gy*W + gx  (exact in f32, < 2^24)
            fc = sp.tile([P, 1], f32)
            nc.vector.tensor_scalar(
                out=fc[:], in0=gy[:], scalar1=float(W), scalar2=None, op0=ALU.mult
            )
            nc.vector.tensor_tensor(out=fc[:], in0=fc[:], in1=gx[:], op=ALU.add)
            nc.vector.tensor_tensor(out=fc[:], in0=fc[:], in1=tg[:, 7:8], op=ALU.add)

            fci = sp.tile([P, 1], i32)
            nc.vector.tensor_copy(out=fci[:], in_=fc[:])

            # ---------------- gather 8 channels at each target cell ---------
            # offsets staged through a gpsimd-written tile: feeding the
            # DVE-written tile to the dynamic-DMA descriptor generator
            # directly crashes the exec unit (observed empirically).
            fcig = sp.tile([P, 1], i32)
            nc.gpsimd.tensor_copy(out=fcig[:], in_=fci[:])
            gat = sp.tile([P, C], f32)
            nc.gpsimd.indirect_dma_start(
                out=gat[:], out_offset=None,
                in_=cl8_in[:],
                in_offset=bass.IndirectOffsetOnAxis(ap=fcig[:, :1], axis=0),
            )

            # ---------------- duplicate resolution (last write wins) --------
            # round-trip fc through DRAM to replicate it along the free dim of
            # every partition (DMA partition-broadcast) instead of a transpose
            nc.sync.dma_start(out=fc_sc[:], in_=fc[:])
            fct = sp.tile([P, P], f32)
            nc.sync.dma_start(out=fct[:], in_=fc_sc[:].to_broadcast((P, P)))
            sel = sp.tile([P, P], f32)
            nc.vector.tensor_tensor(
                out=sel[:], in0=fc[:].to_broadcast([P, P]), in1=fct[:],
                op=ALU.is_equal,
            )
            # keep only strictly-upper entries (j > i): a later slot writing
            # the same cell. row sum == 0 -> this slot is the winner.
            nc.gpsimd.affine_select(
                out=sel[:], in_=sel[:], compare_op=ALU.is_gt, fill=0.0,
                base=0, channel_multiplier=-1, pattern=[[1, P]],
            )
            dup_after = sp.tile([P, 1], f32)
            nc.vector.reduce_sum(
                out=dup_after[:], in_=sel[:], axis=mybir.AxisListType.X
            )
            m = sp.tile([P, 1], f32)
            nc.vector.tensor_scalar(
                out=m[:], in0=dup_after[:], scalar1=0.0, scalar2=None,
                op0=ALU.is_equal,
            )
            nc.vector.tensor_copy(out=out_t[:, COL_M : COL_M + 1], in_=m[:])

            # masked cls logit at the cell
            nc.vector.tensor_tensor(
                out=out_t[:, COL_X : COL_X + 1], in0=m[:], in1=gat[:, 0:1],
                op=ALU.mult,
            )

            # ---------------- smooth-l1 on the 7 reg channels ---------------
            d7 = sp.tile([P, 7], f32)
            nc.vector.tensor_tensor(
                out=d7[:], in0=gat[:, 1:C], in1=tg[:, 0:7], op=ALU.subtract
            )
            ad = sp.tile([P, 7], f32)
            nc.vector.tensor_scalar(
                out=ad[:], in0=d7[:], scalar1=-1.0, scalar2=None, op0=ALU.mult
            )
            nc.vector.tensor_tensor(out=ad[:], in0=ad[:], in1=d7[:], op=ALU.max)
            q = sp.tile([P, 7], f32)
            nc.vector.tensor_tensor(out=q[:], in0=ad[:], in1=ad[:], op=ALU.mult)
            nc.vector.tensor_scalar(
                out=q[:], in0=q[:], scalar1=0.5, scalar2=None, op0=ALU.mult
            )
            lin = sp.tile([P, 7], f32)
            nc.vector.tensor_scalar(
                out=lin[:], in0=ad[:], scalar1=0.5, scalar2=None, op0=ALU.subtract
            )
            lt = sp.tile([P, 7], mybir.dt.uint8)
            nc.vector.tensor_scalar(
                out=lt[:], in0=ad[:], scalar1=1.0, scalar2=None, op0=ALU.is_lt
            )
            sl1 = sp.tile([P, 7], f32)
            nc.vector.select(out=sl1[:], mask=lt[:], on_true=q[:], on_false=lin[:])
            rs = sp.tile([P, 1], f32)
            nc.vector.reduce_sum(out=rs[:], in_=sl1[:], axis=mybir.AxisListType.X)
            nc.vector.tensor_tensor(
                out=out_t[:, COL_REG : COL_REG + 1], in0=rs[:], in1=m[:],
                op=ALU.mult,
            )

            nc.sync.dma_start(out=out_d[:], in_=out_t[:])

    nc.compile()
    return nc


def kernel(preds: np.ndarray, targets: np.ndarray) -> tuple:
    global _compiled
    preds = np.ascontiguousarray(np.asarray(preds, dtype=np.float32))
    targets = np.ascontiguousarray(np.asarray(targets, dtype=np.float32))

    # host-side layout prep (no reductions/FLOPs on tensor data, just copies):
    # contiguous cls channel for the dense pass, channel-last copy so one
    # indirect-DMA row fetches all 8 channels of a cell.
    cls = np.ascontiguousarray(preds[:, 0])                       # (B,H,W)
    cl8 = np.ascontiguousarray(
        np.transpose(preds.reshape(B, C, CELLS), (0, 2, 1))       # (B,CELLS,C)
    )

    if _compiled is None:
        _compiled = _build()
    nc = _compiled

    boff_col = np.repeat(
        np.arange(BPC, dtype=np.float32) * CELLS, N_TGT
    ).reshape(SLOTS, 1)
    in_maps = []
    for c in range(NCORES):
        b0 = c * BPC
        cls2d = cls[b0 : b0 + BPC].reshape(P, FREE)
        tg8 = np.concatenate(
            [targets[b0 : b0 + BPC].reshape(SLOTS, 7), boff_col], axis=1
        )
        in_maps.append({
            "cls": np.ascontiguousarray(np.concatenate(
                [cls2d[:, 0 : SPLITS[0]], tg8, cls2d[:, SPLITS[0] :]], axis=1
            )),
            "cl8": cl8[b0 : b0 + BPC].reshape(CORE_ELEMS, C),
        })

    try:
        res = run_bass_kernel_spmd(nc, in_maps, list(range(NCORES))).results
    except Exception:
        # the axon worker occasionally dies with NRT_EXEC_UNIT_UNRECOVERABLE
        # on arbitrary ops (observed on plain jax PRNG calls too) and recovers
        # on the next attempt; retry once before giving up.
        res = run_bass_kernel_spmd(nc, in_maps, list(range(NCORES))).results

    outs = np.stack([np.asarray(r["out"], dtype=np.float64) for r in res])
    s_softplus = outs[:, :, 0:NCHUNK].sum()
    s_x = outs[:, :, COL_X].sum()
    num_objects = outs[:, :, COL_M].sum()
    s_reg = outs[:, :, COL_REG].sum()

    m_total = float(B * H * W)
    cls_loss = s_softplus / m_total - s_x / m_total
    reg_loss = s_reg / (num_objects + 1e-6) if num_objects > 0 else 0.0
    total = np.float32(cls_loss + 2.0 * reg_loss)
    return total, np.float32(num_objects)

